# revision 11
# baseline (speedup 1.0000x reference)
"""Multi-head self-attention (B=2,S=2048,D=1024,H=16) on 8 NeuronCores.

Sharding: batch x head-group. Core c handles batch c//4, heads [4*(c%4), 4*(c%4)+4).
Each core computes QKV projections for its heads, masked softmax attention, and a
partial output projection (its heads' slice of w_out). Host sums the 4 partials
per batch and adds b_out.

Layout notes (per core):
  xt   [1152, 2048] bf16 : x[b]^T padded; row 1024 = 1.0 (bias row), rest 0.
  wq/wk[1152, 256]  bf16 : w_qkv slice for this core's heads; row 1024 = bias.
  wv   [1152, 260]  bf16 : per local head hl: cols hl*65..hl*65+64 = W_V head cols
                           (+bias row), col hl*65+64 = e_1024 (makes a ones column
                           in V == softmax denominator accumulator).
  Q^T/K^T kept transposed [head*depth, S]; logits computed transposed [k, q] so
  exp(logits) feeds attn@V (contraction over k on partitions) with no transposes.
  Mask folded into exp's per-partition bias (-1e9 * mask), 1/sqrt(64) into scale.
"""

import sys

if "/opt/trn_rl_repo" not in sys.path:
    sys.path.insert(0, "/opt/trn_rl_repo")

import ml_dtypes
import numpy as np

B, S, D, H = 2, 2048, 1024, 16
DEPTH = D // H
NCORES = 8
HLOC = 4            # heads per core
DK = 1152           # padded contraction dim (1024 + bias row, padded to 9*128)
NKT = S // 128      # 16 k-position tiles
DKT = DK // 128     # 9 contraction tiles
BF16 = ml_dtypes.bfloat16

_CACHE: dict = {}


def _build_program(debug_dumps=False):
    import concourse.tile as tile
    from concourse import bacc, mybir

    f32 = mybir.dt.float32
    bf16 = mybir.dt.bfloat16
    Exp = mybir.ActivationFunctionType.Exp

    nc = bacc.Bacc("TRN2", target_bir_lowering=False, debug=False, num_devices=NCORES)

    xt_d = nc.dram_tensor("xt", [DK, S], bf16, kind="ExternalInput")
    wq_d = nc.dram_tensor("wq", [DK, 256], bf16, kind="ExternalInput")
    wk_d = nc.dram_tensor("wk", [DK, 256], bf16, kind="ExternalInput")
    wv_d = nc.dram_tensor("wv", [DK, 260], bf16, kind="ExternalInput")
    wo_d = nc.dram_tensor("wo", [256, D], bf16, kind="ExternalInput")
    mb_d = nc.dram_tensor("mb", [128, NKT], f32, kind="ExternalInput")
    ones_d = nc.dram_tensor("ones", [1, 64], f32, kind="ExternalInput")
    out_d = nc.dram_tensor("out", [S, D], f32, kind="ExternalOutput")
    if debug_dumps:
        dbg_qt = nc.dram_tensor("dbg_qt", [256, S], mybir.dt.bfloat16, kind="ExternalOutput")
        dbg_kt = nc.dram_tensor("dbg_kt", [256, S], mybir.dt.bfloat16, kind="ExternalOutput")
        dbg_v = nc.dram_tensor("dbg_v", [S, 260], mybir.dt.bfloat16, kind="ExternalOutput")
        dbg_pt = nc.dram_tensor("dbg_pt", [2 * NKT * 128, 1024], mybir.dt.bfloat16, kind="ExternalOutput")
        dbg_at = nc.dram_tensor("dbg_at", [256, S], mybir.dt.bfloat16, kind="ExternalOutput")
        dbg_acc = nc.dram_tensor("dbg_acc", [65, 512], f32, kind="ExternalOutput")
        dbg_recip = nc.dram_tensor("dbg_recip", [1, 512], f32, kind="ExternalOutput")
        dbg_rep = nc.dram_tensor("dbg_rep", [64, 512], f32, kind="ExternalOutput")
        dbg_tmp = nc.dram_tensor("dbg_tmp", [64, 512], f32, kind="ExternalOutput")

    with tile.TileContext(nc) as tc:
        with (
            tc.tile_pool(name="w", bufs=1) as pw,
            tc.tile_pool(name="xt", bufs=1) as pxt,
            tc.tile_pool(name="big", bufs=1) as pbig,
            tc.tile_pool(name="pt", bufs=1) as ppt,
            tc.tile_pool(name="tmp", bufs=4) as ptmp,
            tc.tile_pool(name="so", bufs=3) as pso,
            tc.tile_pool(name="pslog", bufs=2, space="PSUM") as psl,
            tc.tile_pool(name="psA", bufs=2, space="PSUM") as psa,
            tc.tile_pool(name="psB", bufs=2, space="PSUM") as psb,
        ):
            # ---- constant / weight loads ----
            wq_sb = [pw.tile([128, 256], bf16, tag=f"wq{k}", name=f"wq{k}") for k in range(DKT)]
            wk_sb = [pw.tile([128, 256], bf16, tag=f"wk{k}", name=f"wk{k}") for k in range(DKT)]
            wv_sb = [pw.tile([128, 260], bf16, tag=f"wv{k}", name=f"wv{k}") for k in range(DKT)]
            wo_sb = [pw.tile([128, D], bf16, tag=f"wo{k}", name=f"wo{k}") for k in range(2)]
            mb_sb = pw.tile([128, NKT], f32, tag="mb", name="mb")
            ones_sb = pw.tile([1, 64], f32, tag="ones", name="ones")
            for k in range(DKT):
                nc.sync.dma_start(wq_sb[k][:], wq_d[128 * k : 128 * (k + 1), :])
                nc.sync.dma_start(wk_sb[k][:], wk_d[128 * k : 128 * (k + 1), :])
                nc.sync.dma_start(wv_sb[k][:], wv_d[128 * k : 128 * (k + 1), :])
            for k in range(2):
                nc.sync.dma_start(wo_sb[k][:], wo_d[128 * k : 128 * (k + 1), :])
            nc.sync.dma_start(mb_sb[:], mb_d[:])
            nc.sync.dma_start(ones_sb[:], ones_d[:])

            xt_sb = [pxt.tile([128, S], bf16, tag=f"xt{k}", name=f"xt{k}") for k in range(DKT)]
            for k in range(DKT):
                nc.sync.dma_start(xt_sb[k][:], xt_d[128 * k : 128 * (k + 1), :])

            # persistent activations
            qt_sb = [pbig.tile([128, S], bf16, tag=f"qt{g}", name=f"qt{g}") for g in range(2)]
            kt_sb = [pbig.tile([128, S], bf16, tag=f"kt{g}", name=f"kt{g}") for g in range(2)]
            v_sb = [pbig.tile([128, 260], bf16, tag=f"v{st}", name=f"v{st}") for st in range(NKT)]
            at_sb = [pbig.tile([128, S], bf16, tag=f"at{g}", name=f"at{g}") for g in range(2)]

            def qk_proj(g, which):
                w_sb = wq_sb if which == "q" else wk_sb
                dst = qt_sb[g] if which == "q" else kt_sb[g]
                for ch in range(4):
                    ps = psb.tile([128, 512], f32, tag="psB", name="psB")
                    for k in range(DKT):
                        nc.tensor.matmul(
                            ps[:],
                            w_sb[k][:, 128 * g : 128 * (g + 1)],
                            xt_sb[k][:, 512 * ch : 512 * (ch + 1)],
                            start=(k == 0),
                            stop=(k == DKT - 1),
                        )
                    nc.vector.tensor_copy(dst[:, 512 * ch : 512 * (ch + 1)], ps[:])

            def v_proj():
                for st in range(NKT):
                    ps = psb.tile([128, 512], f32, tag="psB", name="psB")
                    for k in range(DKT):
                        nc.tensor.matmul(
                            ps[:, 0:260],
                            xt_sb[k][:, 128 * st : 128 * (st + 1)],
                            wv_sb[k][:],
                            start=(k == 0),
                            stop=(k == DKT - 1),
                        )
                    nc.vector.tensor_copy(v_sb[st][:], ps[:, 0:260])

            def stage_a(g, qh):
                """logits + exp for head pair g, query half qh -> pt tiles."""
                pts = [[None] * NKT for _ in range(2)]
                for kt in range(NKT):
                    for e in range(2):
                        ps = psl.tile([128, 1024], f32, tag="pslog", name="pslog")
                        for j in range(2):
                            q0 = 1024 * qh + 512 * j
                            nc.tensor.matmul(
                                ps[:, 512 * j : 512 * (j + 1)],
                                kt_sb[g][64 * e : 64 * (e + 1), 128 * kt : 128 * (kt + 1)],
                                qt_sb[g][64 * e : 64 * (e + 1), q0 : q0 + 512],
                                start=True,
                                stop=True,
                            )
                        pt = ppt.tile([128, 1024], bf16, tag=f"pt{e}_{kt}", name=f"pt{e}_{kt}")
                        nc.scalar.activation(
                            pt[:], ps[:], Exp,
                            bias=mb_sb[:, kt : kt + 1],
                            scale=0.125,
                        )
                        pts[e][kt] = pt
                        if debug_dumps and g == 0 and qh == 0:
                            nc.sync.dma_start(
                                dbg_pt[(e * NKT + kt) * 128 : (e * NKT + kt + 1) * 128, :],
                                pt[:],
                            )
                return pts

            def stage_b(g, qh, pts):
                """attn@V + normalize for head pair g over query half qh."""
                for e in range(2):
                    hl = 2 * g + e
                    for j in range(2):
                        qc = 2 * qh + j
                        acc = psa.tile([65, 512], f32, tag="psA", name="psA")
                        for kt in range(NKT):
                            nc.tensor.matmul(
                                acc[:],
                                v_sb[kt][:, 65 * hl : 65 * (hl + 1)],
                                pts[e][kt][:, 512 * j : 512 * (j + 1)],
                                start=(kt == 0),
                                stop=(kt == NKT - 1),
                            )
                        if debug_dumps and g == 0 and qh == 0 and e == 0 and j == 0:
                            dbga = ptmp.tile([65, 512], f32, tag="dbga", name="dbga")
                            nc.vector.tensor_copy(dbga[:], acc[:])
                            nc.sync.dma_start(dbg_acc[:], dbga[:])
                        den = ptmp.tile([1, 512], f32, tag="den", name="den")
                        nc.vector.tensor_copy(den[:], acc[64:65, :])
                        recip = ptmp.tile([1, 512], f32, tag="recip", name="recip")
                        nc.vector.reciprocal_approx_fast(recip[:], den[:])
                        rep = psb.tile([64, 512], f32, tag="psB", name="psB")
                        nc.tensor.matmul(rep[:], ones_sb[:], recip[:], start=True, stop=True)
                        rep_sb = ptmp.tile([64, 512], f32, tag="repsb", name="repsb")
                        nc.vector.tensor_copy(rep_sb[:], rep[:])
                        nc.vector.tensor_mul(
                            at_sb[g][64 * e : 64 * (e + 1), 512 * qc : 512 * (qc + 1)],
                            acc[0:64, :],
                            rep_sb[:],
                        )
                        if debug_dumps and g == 0 and qh == 0 and e == 0 and j == 0:
                            nc.sync.dma_start(dbg_recip[:], recip[:])
                            pass
                            dbgr = ptmp.tile([64, 512], f32, tag="dbgr", name="dbgr")
                            nc.vector.tensor_copy(dbgr[:], rep[:])
                            nc.sync.dma_start(dbg_rep[:], dbgr[:])

            def out_proj(st_range):
                for st in st_range:
                    for n in range(2):
                        ps = psb.tile([128, 512], f32, tag="psB", name="psB")
                        for ct in range(2):
                            nc.tensor.matmul(
                                ps[:],
                                at_sb[ct][:, 128 * st : 128 * (st + 1)],
                                wo_sb[ct][:, 512 * n : 512 * (n + 1)],
                                start=(ct == 0),
                                stop=(ct == 1),
                            )
                        so = pso.tile([128, 512], f32, tag="so", name="so")
                        nc.vector.tensor_copy(so[:], ps[:])
                        nc.sync.dma_start(
                            out_d[128 * st : 128 * (st + 1), 512 * n : 512 * (n + 1)],
                            so[:],
                        )

            # ---- emission order (scheduling priority) ----
            qk_proj(0, "q")
            qk_proj(0, "k")
            pts = stage_a(0, 0)
            v_proj()
            qk_proj(1, "q")
            qk_proj(1, "k")
            stage_b(0, 0, pts)
            pts = stage_a(0, 1)
            stage_b(0, 1, pts)
            pts = stage_a(1, 0)
            stage_b(1, 0, pts)
            out_proj(range(0, 8))
            pts = stage_a(1, 1)
            stage_b(1, 1, pts)
            out_proj(range(8, 16))
            if debug_dumps:
                for g in range(2):
                    nc.sync.dma_start(dbg_qt[128 * g : 128 * (g + 1), :], qt_sb[g][:])
                    nc.sync.dma_start(dbg_kt[128 * g : 128 * (g + 1), :], kt_sb[g][:])
                    nc.sync.dma_start(dbg_at[128 * g : 128 * (g + 1), :], at_sb[g][:])
                for st in range(NKT):
                    nc.sync.dma_start(dbg_v[128 * st : 128 * (st + 1), :], v_sb[st][:])

    nc.compile()
    return nc


def _get_nc():
    if "nc" not in _CACHE:
        _CACHE["nc"] = _build_program()
    return _CACHE["nc"]


def _prep_core_inputs(c, x, maskf, w_qkv, b_qkv, w_out):
    b, g = c // 4, c % 4
    cq = 256 * g            # col offset of this core's heads in W_Q
    xt = np.zeros((DK, S), dtype=BF16)
    xt[:D, :] = x[b].T.astype(BF16)
    xt[D, :] = BF16(1.0)

    def wslice(col0, ncols=256):
        w = np.zeros((DK, ncols), dtype=BF16)
        w[:D, :] = w_qkv[:, col0 : col0 + ncols].astype(BF16)
        w[D, :] = b_qkv[col0 : col0 + ncols].astype(BF16)
        return w

    wq = wslice(cq)
    wk = wslice(D + cq)
    wv = np.zeros((DK, 260), dtype=BF16)
    for hl in range(HLOC):
        c0 = 2 * D + cq + 64 * hl
        wv[:D, 65 * hl : 65 * hl + 64] = w_qkv[:, c0 : c0 + 64].astype(BF16)
        wv[D, 65 * hl : 65 * hl + 64] = b_qkv[c0 : c0 + 64].astype(BF16)
        wv[D, 65 * hl + 64] = BF16(1.0)
    wo = w_out[cq : cq + 256, :].astype(BF16)
    mb = np.ascontiguousarray(maskf[b].reshape(NKT, 128).T)
    ones = np.ones((1, 64), dtype=np.float32)
    return {"xt": xt, "wq": wq, "wk": wk, "wv": wv, "wo": wo, "mb": mb, "ones": ones}


def kernel(x, mask, w_qkv, b_qkv, w_out, b_out):
    from concourse.bass_utils import run_bass_kernel_spmd

    x = np.asarray(x, dtype=np.float32)
    mask = np.asarray(mask)
    w_qkv = np.asarray(w_qkv, dtype=np.float32)
    b_qkv = np.asarray(b_qkv, dtype=np.float32)
    w_out = np.asarray(w_out, dtype=np.float32)
    b_out = np.asarray(b_out, dtype=np.float32)

    maskf = mask.reshape(B, S).astype(np.float32) * np.float32(-1e9)

    nc = _get_nc()
    in_maps = [
        _prep_core_inputs(c, x, maskf, w_qkv, b_qkv, w_out) for c in range(NCORES)
    ]
    res = run_bass_kernel_spmd(nc, in_maps, core_ids=list(range(NCORES)), **_CACHE.get("run_kwargs", {}))
    _CACHE["last_result"] = res

    out = np.zeros((B, S, D), dtype=np.float32)
    for b in range(B):
        acc = np.zeros((S, D), dtype=np.float32)
        for g in range(4):
            acc += res.results[4 * b + g]["out"]
        out[b] = acc + b_out[None, :]
    return out
